# revision 50
# baseline (speedup 1.0000x reference)
"""Trainium2 Bass kernel: multi-head attention (B=4, S=2048, E=1024, H=16, D=64).

Sharding: 8 cores = 4 batches x 2 head-groups (8 heads each). Each core
computes attention for its (batch, 8-head group) and a partial output
projection over its 512 channels; the host sums the two partials per batch
and adds the output bias.

Per-core dataflow (bf16 matmuls, fp32 PSUM):
  Phase 1 (all heads): A[j] = wA[j].T @ XT_aug[j]
        = [Q'^T (rows 0:64) ; K^T (rows 64:128)], Q' scaled by log2(e)/64;
        B[j] = partition-swapped A[j] (SBUF->SBUF DMA); V_aug tiles.
        All xt loads issued upfront on the SP HWDGE ring; weights and
        B-swaps ride the Act HWDGE ring so input DMA never starves PE.
  Phase 2 (global 3-ahead unit pipeline, 256 units of 2 t-tiles x 512 s):
    unit k: fill scores^T via dual-subarray row tiling (K=64):
        T0 (0,0):  B[0:64,tcols].T @ A[0:64,s]
        T8 (64,0): A[64:128,tcols].T @ B[64:128,s]
    exp(s/8) = 2^(8w): 5 of 8 units on ScalarE (Exp, scale 8*ln2);
        3 on DVE custom 8-stage op (cubic 2^w + 2 squarings -> 2^(4w)),
        final square split: low half DVE, high half GpSimd.
    av(k-3): av += V_aug[tt].T @ exp. Even heads accumulate at psum
        partitions 0:65 (denominator row 64); odd heads at 63:128
        (denominator row 63) so normalized outputs land directly in
        ct rows 64:128 with no partition-shift bounce DMA.
    deferred actions (evict av->oT, denom gather, recip, broadcast,
        GpSimd normalize) fire at fixed unit slots so no engine queue
        ever blocks on a DMA round trip.
  Phase 3: out partial = concatT.T @ WoT -> [s,1024] fp32 -> DRAM
"""

import sys

sys.path.insert(0, "/opt/trn_rl_repo")

import numpy as np
import ml_dtypes

BF16 = ml_dtypes.bfloat16

B, S, E, H = 4, 2048, 1024, 16
D = E // H          # 64
HL = 8              # heads per core
N_CORES = 8
NT = S // 128       # 16 t-tiles
NU = 8              # units per chunk (2 t-tiles each)
NC_CHUNK = 4        # s-chunks of 512
LAG = 7             # av trails fill by LAG units (covers exp latency)
SIG = float(np.log2(np.e) / 64.0)   # score scale folded into Wq

# engine per unit within a chunk: 5 ScalarE / 3 DVE
EU = ["act", "dve", "act", "act", "dve", "act", "dve", "act"]

EXP4_NAME = "ANT_EXP4_SOFTMAX"
EXP4_C3 = 0.0558784277
EXP4_C2 = 0.242307174
EXP4_C1 = 0.693163145
EXP4_A0 = 0.99994823

_CACHE = {}


def register_exp4():
    import concourse.dve_ops as dmod
    from concourse.dve_spec import Spec, Src0, C0, C1, C2, One, sq, lower
    from concourse.dve_uop import DveOpSpec

    for op in dmod.OPS:
        if op.name == EXP4_NAME:
            return op

    body = sq(sq(((C0 * Src0 + C1) * Src0 + C2) * Src0 + One))

    def _ref(in0, in1, s0, s1, imm2):
        x = in0.astype(np.float32)
        p = (((s0 * x + s1) * x + imm2) * x + np.float32(1.0)).astype(np.float32)
        p2 = (p * p).astype(np.float32)
        return (p2 * p2).astype(np.float32)

    spec = Spec(body=body, reference=_ref)
    row = dmod._CUSTOM_DVE_ROW_BASE + len(dmod.OPS)
    assert row < 0x20
    shas = {}
    for ver in ("v3", "v4"):
        try:
            uops = lower(spec, ver=ver)
            shas[ver] = DveOpSpec(
                name=EXP4_NAME, opcode=row, uops=uops, rd1_en=False
            ).sha(ver)
        except Exception:
            pass
    op = dmod.DveOp(EXP4_NAME, spec, subdim=False, uops_sha=shas)
    dmod.OPS.append(op)
    dmod.CUSTOM_DVE_SPECS[EXP4_NAME] = spec
    dmod._SUB_OPCODE_FOR_NAME[EXP4_NAME] = row
    return op


def build_nc():
    import concourse.mybir as mybir
    import concourse.tile as tile
    from concourse import bacc

    f32 = mybir.dt.float32
    bf16 = mybir.dt.bfloat16
    exp4 = register_exp4()
    ACT_SCALE = float(8.0 * np.log(2.0))

    nc = bacc.Bacc(None)

    xt_d = nc.dram_tensor("xt", [HL, 128, S], bf16, kind="ExternalInput")
    # all per-head QKV weights packed into one tensor: one DMA
    wqkv_d = nc.dram_tensor("wqkv", [128, HL * 256], bf16,
                            kind="ExternalInput")
    wot_d = nc.dram_tensor("wot", [HL * D, E], bf16, kind="ExternalInput")
    out_d = nc.dram_tensor("out", [S, E], bf16, kind="ExternalOutput")
    recip_d = nc.dram_tensor("recip_dram", [HL, S], f32)

    # ct[p] rows 0:64 belong to head 2p+1 and rows 64:128 to head 2p
    # (Wo rows permuted to match host-side). Odd heads' normalize then
    # writes ct rows 0:64 directly (no partition-shift bounce); even
    # heads bounce through a [64,*] tile + SBUF DMA, latency-hidden
    # mid-kernel. The last head (7, odd) sits on the phase-3 critical
    # path, so the tail stays short.
    def ct_rows(j):
        return (64, 128) if j % 2 == 0 else (0, 64)

    with tile.TileContext(nc) as tc:
        with (
            tc.tile_pool(name="ab", bufs=2 * HL) as ab_pool,
            tc.tile_pool(name="v", bufs=HL) as v_pool,
            tc.tile_pool(name="wot", bufs=1) as wot_pool,
            tc.tile_pool(name="ct", bufs=4) as ct_pool,
        ):
            As = [None] * HL
            Bs = [None] * HL
            Vs = [None] * HL
            cts = [ct_pool.tile([128, S], bf16, tag="ct", name=f"ct{p}")
                   for p in range(HL // 2)]

            # ---- phase 1: QKV for all heads ----
            with (
                tc.tile_pool(name="xt", bufs=HL) as xt_pool,
                tc.tile_pool(name="w", bufs=1) as w_pool,
                tc.tile_pool(name="qkv_ps", bufs=8, space="PSUM") as qkv_ps,
            ):
                # Input DMAs upfront, spread across queues: xt stream on
                # the SP ring; one combined weights DMA + wot on the Act
                # ring; B-swaps later on the Pool SWDGE queue. Every
                # dma_start costs ~600ns of issuing-engine time, so
                # counts are minimized.
                xts = []
                for j in range(HL):
                    xtj = xt_pool.tile([128, S], bf16, tag="xt",
                                       name=f"xt{j}")
                    nc.sync.dma_start(out=xtj[:, :], in_=xt_d[j])
                    xts.append(xtj)
                WSTR = 256
                wq_t = w_pool.tile([128, HL * WSTR], bf16, tag="w",
                                   name="wqkv")
                nc.scalar.dma_start(out=wq_t[:, :], in_=wqkv_d[:, :])
                def was(j):
                    return wq_t[:, j * WSTR: j * WSTR + 128]

                def wvs(j):
                    return wq_t[:, j * WSTR + 128: j * WSTR + 128 + D + 1]
                wot_t = wot_pool.tile([128, 4 * E], bf16, tag="wot",
                                      name="wot")
                for p in range(4):
                    nc.scalar.dma_start(
                        out=wot_t[:, p * E:(p + 1) * E],
                        in_=wot_d[p * 128:(p + 1) * 128, :])
                

                def emit_a(j):
                    # two half-size psum tiles per head -> 4 slots in
                    # flight, so the 32 A-matmuls run as one dense burst
                    # (keeps the HAM clock gate open) with the psum->bf16
                    # copies trailing on both engines.
                    xtj = xts[j]
                    As[j] = ab_pool.tile([128, S], bf16, tag="ab",
                                         name=f"A{j}")
                    for hf in range(4):
                        ap = qkv_ps.tile([128, 512], f32, tag="qkv",
                                         name=f"ap{j}_{hf}")
                        o = hf * 512
                        nc.tensor.matmul(ap[:, :], was(j),
                                         xtj[:, o:o + 512])
                        if hf % 2 == 0:
                            nc.scalar.copy(As[j][:, o:o + 512], ap[:, :])
                        else:
                            nc.vector.tensor_copy(As[j][:, o:o + 512],
                                                  ap[:, :])
                    Bs[j] = ab_pool.tile([128, S], bf16, tag="ab",
                                         name=f"B{j}")
                    nc.sync.dma_start(out=Bs[j][0:64, :],
                                      in_=As[j][64:128, :])
                    nc.sync.dma_start(out=Bs[j][64:128, :],
                                      in_=As[j][0:64, :])

                def emit_v(j):
                    xtj = xts[j]
                    # V psums: 7 tiles of 65 cols per 512-col bank so no
                    # matmul output crosses a PSUM bank boundary.
                    Vs[j] = v_pool.tile([128, NT * (D + 1)], bf16,
                                        tag="v", name=f"V{j}")
                    vps = [qkv_ps.tile([128, 512], f32, tag="qkv",
                                       name=f"vp{j}_{b}") for b in range(3)]
                    for tt in range(NT):
                        vp = vps[tt // 7]
                        dst = vp[:, (tt % 7) * 65:(tt % 7) * 65 + 65]
                        nc.tensor.matmul(
                            dst, xtj[:, tt * 128:(tt + 1) * 128], wvs(j))
                    for bk in range(3):
                        nb = min(7, NT - bk * 7)
                        dst = Vs[j][:, bk * 7 * (D + 1):
                                    (bk * 7 + nb) * (D + 1)]
                        src = vps[bk][:, 0:nb * 65]
                        # alternate engines per block so each head's
                        # three V-copies split across ScalarE and DVE
                        if (bk + j) % 2 == 0:
                            nc.vector.tensor_copy(dst, src)
                        else:
                            nc.scalar.copy(dst, src)

                # All A matmuls first: dense N=512 streams un-throttle
                # the PE HAM clock gate early; the V block follows.
                for j in range(HL):
                    emit_a(j)
                for j in range(HL):
                    emit_v(j)

            # ---- phase 2: attention, global 3-ahead pipeline ----
            with (
                tc.tile_pool(name="psR", bufs=3, space="PSUM") as psR,
                tc.tile_pool(name="psav", bufs=2, space="PSUM") as psav,
                tc.tile_pool(name="eg", bufs=9) as eg_pool,
                tc.tile_pool(name="egh", bufs=10) as egh_pool,
                tc.tile_pool(name="eh", bufs=4) as eh_pool,
                tc.tile_pool(name="ot", bufs=2) as ot_pool,
                tc.tile_pool(name="norm", bufs=4) as norm_pool,
            ):
                NCH = HL * NC_CHUNK          # 32 global chunks
                K = NCH * NU                 # 256 global units

                oTs = {}                     # head -> oT tile
                avs = {}                     # chunk -> av tile
                egs = {}                     # unit -> (eg_lo_ap, eg_hi_ap)
                avn = {}                     # chunk -> accumulated count
                csd = {}
                bcd = {}
                sched = {}                   # global unit idx -> [closures]

                def at(k, fn):
                    sched.setdefault(k, []).append(fn)

                def fill_unit(k):
                    j, r = divmod(k, NC_CHUNK * NU)
                    c, u = divmod(r, NU)
                    sl = slice(c * 512, (c + 1) * 512)
                    gt = psR.tile([128, 1024], f32, tag="psR",
                                  name=f"g{k}")
                    ta, tb = 2 * u, 2 * u + 1
                    nc.tensor.matmul(
                        gt[:, 0:512],
                        Bs[j][0:64, ta * 128:(ta + 1) * 128],
                        As[j][0:64, sl], tile_position=(0, 0))
                    nc.tensor.matmul(
                        gt[:, 512:1024],
                        As[j][64:128, tb * 128:(tb + 1) * 128],
                        Bs[j][64:128, sl], tile_position=(64, 0))
                    return gt

                def emit_exp(k, gt):
                    j, r = divmod(k, NC_CHUNK * NU)
                    c, u = divmod(r, NU)
                    if EU[u] == "act":
                        eg = eg_pool.tile([128, 1024], bf16, tag="eg",
                                          name=f"eg{k}")
                        nc.scalar.activation(
                            eg[:, :], gt[:, :],
                            mybir.ActivationFunctionType.Exp,
                            scale=ACT_SCALE)
                        egs[k] = (eg[:, 0:512], eg[:, 512:1024])
                    else:
                        eh = eh_pool.tile([128, 1024], bf16, tag="eh",
                                          name=f"eh{k}")
                        nc.vector._custom_dve(
                            exp4, out=eh[:, :], in0=gt[:, :],
                            s0=EXP4_C3, s1=EXP4_C2, imm2=EXP4_C1)
                        elo = egh_pool.tile([128, 512], bf16,
                                            tag="egh", name=f"egl{k}")
                        ehi = egh_pool.tile([128, 512], bf16,
                                            tag="egh", name=f"egh{k}")
                        nc.vector.tensor_tensor(
                            elo[:, :], eh[:, 0:512], eh[:, 0:512],
                            op=mybir.AluOpType.mult)
                        nc.gpsimd.tensor_tensor(
                            ehi[:, :], eh[:, 512:1024], eh[:, 512:1024],
                            op=mybir.AluOpType.mult)
                        egs[k] = (elo[:, :], ehi[:, :])

                def emit_av(k):
                    j, r = divmod(k, NC_CHUNK * NU)
                    c, u = divmod(r, NU)
                    m = k // NU              # global chunk
                    if m not in avs:
                        avs[m] = psav.tile([D + 1, 512], f32, tag="psav",
                                           name=f"av{m}")
                        avn[m] = 0
                    av = avs[m]
                    eg_lo, eg_hi = egs.pop(k)
                    for half, egp in ((0, eg_lo), (1, eg_hi)):
                        tt = 2 * u + half
                        nc.tensor.matmul(
                            av[:, :],
                            Vs[j][:, tt * (D + 1):(tt + 1) * (D + 1)],
                            egp,
                            start=(avn[m] == 0),
                            stop=(avn[m] == NT - 1))
                        avn[m] += 1

                def queue_chunk_end(m):
                    """Called right after emit_av completes chunk m, at
                    global unit k0 = (m+1)*NU + LAG - 1."""
                    j, c = divmod(m, NC_CHUNK)
                    k0 = (m + 1) * NU + LAG - 1
                    if j == HL - 1:
                        # last head: fast per-chunk norm path (chunk-sized
                        # denominator round trips so the tail is short)
                        emit_evict(m)
                        emit_cs(j, c * 512, 512)
                        if c > 0:
                            s0 = (c - 1) * 512
                            at(k0 + 1,
                               lambda: emit_recip_bcast(j, s0, 512))
                            at(k0 + 3, lambda: emit_mult(j, s0, s0, 512))
                        return
                    at(k0 + 2, lambda: emit_evict(m))
                    if c % 2 == 1:
                        h0 = (c - 1) * 512
                        at(k0 + 3, lambda: emit_cs(j, h0, 1024))
                        at(k0 + 6, lambda: emit_recip_bcast(j, h0, 1024))
                        at(k0 + 14, lambda: emit_mult(j, h0, h0, 512))
                        at(k0 + 16,
                           lambda: emit_mult(j, h0, h0 + 512, 512))

                def emit_evict(m):
                    j, c = divmod(m, NC_CHUNK)
                    if j not in oTs:
                        oTs[j] = ot_pool.tile([D + 1, S], f32, tag="ot",
                                              name=f"oT{j}")
                    av = avs.pop(m)
                    del avn[m]
                    if m % 2 == 0:
                        nc.vector.tensor_copy(
                            oTs[j][:, c * 512:(c + 1) * 512], av[:, :])
                    else:
                        nc.scalar.copy(
                            oTs[j][:, c * 512:(c + 1) * 512], av[:, :])

                def emit_cs(j, s0, n):
                    cs = norm_pool.tile([128, 16], f32, tag="cs",
                                        name=f"cs{j}_{s0}")
                    nc.sync.dma_start(out=cs[:, 0:n // 128],
                                      in_=oTs[j][D:D + 1, s0:s0 + n])
                    csd[(j, s0)] = cs

                def emit_recip_bcast(j, s0, n):
                    cs = csd.pop((j, s0))
                    rc = norm_pool.tile([128, 16], f32, tag="rc",
                                        name=f"rc{j}_{s0}")
                    nc.vector.reciprocal(rc[:, 0:n // 128], cs[:, 0:n // 128])
                    nc.sync.dma_start(out=recip_d[j, s0:s0 + n],
                                      in_=rc[:, 0:n // 128])
                    bc = norm_pool.tile([D, 1024], f32, tag="bc",
                                        name=f"bc{j}_{s0}")
                    nc.sync.dma_start(
                        out=bc[:, 0:n],
                        in_=recip_d[j, s0:s0 + n].unsqueeze(0)
                        .broadcast_to((D, n)))
                    bcd[(j, s0)] = bc

                def emit_mult(j, h0, s0, n):
                    bc = bcd[(j, h0)]
                    boff = s0 - h0
                    ct = cts[j // 2]
                    if j % 2 == 1:
                        # odd head: ct rows 0:64, direct GpSimd write
                        nc.gpsimd.tensor_tensor(
                            ct[0:D, s0:s0 + n],
                            oTs[j][0:D, s0:s0 + n],
                            bc[:, boff:boff + n],
                            op=mybir.AluOpType.mult)
                    else:
                        # even head: bounce via [64,*] tile + SBUF DMA
                        # into ct rows 64:128 (partition shift)
                        dst = norm_pool.tile([D, 1024], bf16, tag="ctmp",
                                             name=f"ctmp{j}_{s0}")
                        nc.gpsimd.tensor_tensor(
                            dst[:, 0:n],
                            oTs[j][0:D, s0:s0 + n],
                            bc[:, boff:boff + n],
                            op=mybir.AluOpType.mult)
                        nc.sync.dma_start(
                            out=ct[D:2 * D, s0:s0 + n], in_=dst[:, 0:n])

                for k in range(K + LAG):
                    if k < K:
                        gt = fill_unit(k)
                        emit_exp(k, gt)
                    ka = k - LAG
                    if ka >= 0:
                        emit_av(ka)
                        if ka % NU == NU - 1:
                            queue_chunk_end(ka // NU)
                    for fn in sched.pop(k, []):
                        fn()

                # drain remaining scheduled work, then the final chunk's
                # normalization for the last head.
                for k in sorted(sched):
                    for fn in sched[k]:
                        fn()
                sched.clear()
                j = HL - 1
                emit_recip_bcast(j, 3 * 512, 512)
                emit_mult(j, 3 * 512, 3 * 512, 512)

            # ---- phase 3: output projection ----
            with (
                tc.tile_pool(name="pj_ps", bufs=4, space="PSUM") as pj_ps,
                tc.tile_pool(name="po", bufs=4) as po_pool,
            ):
                NS = S // 128
                psos = {}

                def pj_front(sc):
                    pso = pj_ps.tile([128, E], f32, tag="pj",
                                     name=f"pso{sc}")
                    psos[sc] = pso
                    for p in range(3):
                        for half in range(2):
                            hsl = slice(half * 512, (half + 1) * 512)
                            nc.tensor.matmul(
                                pso[:, hsl],
                                cts[p][:, sc * 128:(sc + 1) * 128],
                                wot_t[:, p * E + half * 512:
                                      p * E + (half + 1) * 512],
                                start=(p == 0), stop=False)

                def pj_back(sc):
                    pso = psos.pop(sc)
                    for half in range(2):
                        hsl = slice(half * 512, (half + 1) * 512)
                        nc.tensor.matmul(
                            pso[:, hsl],
                            cts[3][:, sc * 128:(sc + 1) * 128],
                            wot_t[:, 3 * E + half * 512:
                                  3 * E + (half + 1) * 512],
                            start=False, stop=True)
                    # bf16 partial output: halves the out-DMA traffic.
                    # Separate even/odd tile tags keep the vector and
                    # scalar copy chains independent.
                    if sc % 2 == 0:
                        osb = po_pool.tile([128, E], bf16, tag="pov")
                        nc.vector.tensor_copy(osb[:, :], pso[:, :])
                    else:
                        osb = po_pool.tile([128, E], bf16, tag="pos")
                        nc.scalar.copy(osb[:, :], pso[:, :])
                    nc.sync.dma_start(out=out_d[sc * 128:(sc + 1) * 128, :],
                                      in_=osb[:, :])

                for sc in range(NS + 4):
                    if sc < NS:
                        pj_front(sc)
                    if sc >= 4:
                        pj_back(sc - 4)

    nc.compile()
    return nc


def prep_inputs(token_encodings, Wq, Wk, Wv, bq, bk, bv, Wo, bo):
    """Build per-core input maps. Core c = b*2+g."""
    x = np.asarray(token_encodings, dtype=np.float32)
    wq = np.asarray(Wq, np.float32)
    wk = np.asarray(Wk, np.float32)
    wv = np.asarray(Wv, np.float32)
    bq_ = np.asarray(bq, np.float32)
    bk_ = np.asarray(bk, np.float32)
    bv_ = np.asarray(bv, np.float32)
    wo = np.asarray(Wo, np.float32)
    maps = []
    for c in range(N_CORES):
        b, g = divmod(c, 2)
        xt_full = np.ascontiguousarray(x[b].T)  # (E, S)
        xt = np.zeros((HL, 128, S), dtype=BF16)
        WSTR = 256
        wqkv = np.zeros((128, HL * WSTR), dtype=BF16)
        for j in range(HL):
            h = g * HL + j
            xt[j, :D] = xt_full[h * D:(h + 1) * D].astype(BF16)
            xt[j, D] = np.float32(1.0)
            o = j * WSTR
            # A-stationary: cols 0:64 -> Q' (scaled), cols 64:128 -> K
            wqkv[:D, o:o + D] = (wq[h] * SIG).astype(BF16)
            wqkv[D, o:o + D] = (bq_[h] * SIG).astype(BF16)
            wqkv[:D, o + D:o + 2 * D] = wk[h].astype(BF16)
            wqkv[D, o + D:o + 2 * D] = bk_[h].astype(BF16)
            # V-stationary: cols 128:192 -> V, col 192 -> denominator
            wqkv[:D, o + 128:o + 128 + D] = wv[h].astype(BF16)
            wqkv[D, o + 128:o + 128 + D] = bv_[h].astype(BF16)
            wqkv[D, o + 128 + D] = np.float32(1.0)
        # wot rows permuted: ct[p] rows 0:64 = head 2p+1, 64:128 = head 2p
        order = []
        for p in range(4):
            for r in range(128):
                j_loc = 2 * p + 1 if r < 64 else 2 * p
                order.append(g * 512 + j_loc * D + (r % D))
        wot = np.ascontiguousarray(wo[:, order].T).astype(BF16)
        maps.append({"xt": xt, "wqkv": wqkv, "wot": wot})
    return maps


def kernel(**inputs):
    from concourse.bass_utils import run_bass_kernel_spmd

    if "nc" not in _CACHE:
        _CACHE["nc"] = build_nc()
    nc = _CACHE["nc"]
    in_maps = prep_inputs(**inputs)
    res = run_bass_kernel_spmd(nc, in_maps, list(range(N_CORES)))
    bo_f = np.asarray(inputs["bo"], np.float32)
    out = np.empty((B, S, E), dtype=np.float32)
    for b in range(B):
        out[b] = (res.results[2 * b]["out"].astype(np.float32)
                  + res.results[2 * b + 1]["out"].astype(np.float32)
                  + bo_f)
    return out


# revision 51
# speedup vs baseline: 1.0266x; 1.0266x over previous
"""Trainium2 Bass kernel: multi-head attention (B=4, S=2048, E=1024, H=16, D=64).

Sharding: 8 cores = 4 batches x 2 head-groups (8 heads each). Each core
computes attention for its (batch, 8-head group) and a partial output
projection over its 512 channels; the host sums the two partials per batch
and adds the output bias.

Per-core dataflow (bf16 matmuls, fp32 PSUM):
  Phase 1 (all heads): A[j] = wA[j].T @ XT_aug[j]
        = [Q'^T (rows 0:64) ; K^T (rows 64:128)], Q' scaled by log2(e)/64;
        B[j] = partition-swapped A[j] (SBUF->SBUF DMA); V_aug tiles.
        All xt loads issued upfront on the SP HWDGE ring; weights and
        B-swaps ride the Act HWDGE ring so input DMA never starves PE.
  Phase 2 (global 3-ahead unit pipeline, 256 units of 2 t-tiles x 512 s):
    unit k: fill scores^T via dual-subarray row tiling (K=64):
        T0 (0,0):  B[0:64,tcols].T @ A[0:64,s]
        T8 (64,0): A[64:128,tcols].T @ B[64:128,s]
    exp(s/8) = 2^(8w): 5 of 8 units on ScalarE (Exp, scale 8*ln2);
        3 on DVE custom 8-stage op (cubic 2^w + 2 squarings -> 2^(4w)),
        final square split: low half DVE, high half GpSimd.
    av(k-3): av += V_aug[tt].T @ exp. Even heads accumulate at psum
        partitions 0:65 (denominator row 64); odd heads at 63:128
        (denominator row 63) so normalized outputs land directly in
        ct rows 64:128 with no partition-shift bounce DMA.
    deferred actions (evict av->oT, denom gather, recip, broadcast,
        GpSimd normalize) fire at fixed unit slots so no engine queue
        ever blocks on a DMA round trip.
  Phase 3: out partial = concatT.T @ WoT -> [s,1024] fp32 -> DRAM
"""

import sys

sys.path.insert(0, "/opt/trn_rl_repo")

import numpy as np
import ml_dtypes

BF16 = ml_dtypes.bfloat16

B, S, E, H = 4, 2048, 1024, 16
D = E // H          # 64
HL = 8              # heads per core
N_CORES = 8
NT = S // 128       # 16 t-tiles
NU = 8              # units per chunk (2 t-tiles each)
NC_CHUNK = 4        # s-chunks of 512
LAG = 7             # av trails fill by LAG units (covers exp latency)
SIG = float(np.log2(np.e) / 64.0)   # score scale folded into Wq

# engine per unit within a chunk: 5 ScalarE / 3 DVE
EU = ["act", "dve", "act", "act", "dve", "act", "dve", "act"]

EXP4_NAME = "ANT_EXP4_SOFTMAX"
EXP4_C3 = 0.0558784277
EXP4_C2 = 0.242307174
EXP4_C1 = 0.693163145
EXP4_A0 = 0.99994823

_CACHE = {}


def register_exp4():
    import concourse.dve_ops as dmod
    from concourse.dve_spec import Spec, Src0, C0, C1, C2, One, sq, lower
    from concourse.dve_uop import DveOpSpec

    for op in dmod.OPS:
        if op.name == EXP4_NAME:
            return op

    body = sq(sq(((C0 * Src0 + C1) * Src0 + C2) * Src0 + One))

    def _ref(in0, in1, s0, s1, imm2):
        x = in0.astype(np.float32)
        p = (((s0 * x + s1) * x + imm2) * x + np.float32(1.0)).astype(np.float32)
        p2 = (p * p).astype(np.float32)
        return (p2 * p2).astype(np.float32)

    spec = Spec(body=body, reference=_ref)
    row = dmod._CUSTOM_DVE_ROW_BASE + len(dmod.OPS)
    assert row < 0x20
    shas = {}
    for ver in ("v3", "v4"):
        try:
            uops = lower(spec, ver=ver)
            shas[ver] = DveOpSpec(
                name=EXP4_NAME, opcode=row, uops=uops, rd1_en=False
            ).sha(ver)
        except Exception:
            pass
    op = dmod.DveOp(EXP4_NAME, spec, subdim=False, uops_sha=shas)
    dmod.OPS.append(op)
    dmod.CUSTOM_DVE_SPECS[EXP4_NAME] = spec
    dmod._SUB_OPCODE_FOR_NAME[EXP4_NAME] = row
    return op


def build_nc():
    import concourse.mybir as mybir
    import concourse.tile as tile
    from concourse import bacc

    f32 = mybir.dt.float32
    bf16 = mybir.dt.bfloat16
    exp4 = register_exp4()
    ACT_SCALE = float(8.0 * np.log(2.0))

    nc = bacc.Bacc(None)

    xt_d = nc.dram_tensor("xt", [HL, 128, S], bf16, kind="ExternalInput")
    # all per-head QKV weights packed into one tensor: one DMA
    wqkv_d = nc.dram_tensor("wqkv", [128, HL * 256], bf16,
                            kind="ExternalInput")
    wot_d = nc.dram_tensor("wot", [HL * D, E], bf16, kind="ExternalInput")
    out_d = nc.dram_tensor("out", [S, E], bf16, kind="ExternalOutput")
    recip_d = nc.dram_tensor("recip_dram", [HL, S], f32)

    # ct[p] rows 0:64 belong to head 2p+1 and rows 64:128 to head 2p
    # (Wo rows permuted to match host-side). Odd heads' normalize then
    # writes ct rows 0:64 directly (no partition-shift bounce); even
    # heads bounce through a [64,*] tile + SBUF DMA, latency-hidden
    # mid-kernel. The last head (7, odd) sits on the phase-3 critical
    # path, so the tail stays short.
    def ct_rows(j):
        return (64, 128) if j % 2 == 0 else (0, 64)

    with tile.TileContext(nc) as tc:
        with (
            tc.tile_pool(name="ab", bufs=2 * HL) as ab_pool,
            tc.tile_pool(name="v", bufs=HL) as v_pool,
            tc.tile_pool(name="wot", bufs=1) as wot_pool,
            tc.tile_pool(name="ct", bufs=4) as ct_pool,
        ):
            As = [None] * HL
            Bs = [None] * HL
            Vs = [None] * HL
            cts = [ct_pool.tile([128, S], bf16, tag="ct", name=f"ct{p}")
                   for p in range(HL // 2)]

            # ---- phase 1: QKV for all heads ----
            with (
                tc.tile_pool(name="xt", bufs=HL) as xt_pool,
                tc.tile_pool(name="w", bufs=1) as w_pool,
                tc.tile_pool(name="qkv_ps", bufs=8, space="PSUM") as qkv_ps,
            ):
                # Input DMAs upfront, spread across queues: xt stream on
                # the SP ring; one combined weights DMA + wot on the Act
                # ring; B-swaps later on the Pool SWDGE queue. Every
                # dma_start costs ~600ns of issuing-engine time, so
                # counts are minimized.
                xts = []
                for j in range(HL):
                    xtj = xt_pool.tile([128, S], bf16, tag="xt",
                                       name=f"xt{j}")
                    nc.sync.dma_start(out=xtj[:, :], in_=xt_d[j])
                    xts.append(xtj)
                WSTR = 256
                wq_t = w_pool.tile([128, HL * WSTR], bf16, tag="w",
                                   name="wqkv")
                nc.scalar.dma_start(out=wq_t[:, :], in_=wqkv_d[:, :])
                def was(j):
                    return wq_t[:, j * WSTR: j * WSTR + 128]

                def wvs(j):
                    return wq_t[:, j * WSTR + 128: j * WSTR + 128 + D + 1]
                wot_t = wot_pool.tile([128, 4 * E], bf16, tag="wot",
                                      name="wot")
                for p in range(4):
                    nc.scalar.dma_start(
                        out=wot_t[:, p * E:(p + 1) * E],
                        in_=wot_d[p * 128:(p + 1) * 128, :])
                

                def emit_a(j):
                    # two half-size psum tiles per head -> 4 slots in
                    # flight, so the 32 A-matmuls run as one dense burst
                    # (keeps the HAM clock gate open) with the psum->bf16
                    # copies trailing on both engines.
                    xtj = xts[j]
                    As[j] = ab_pool.tile([128, S], bf16, tag="ab",
                                         name=f"A{j}")
                    for hf in range(4):
                        ap = qkv_ps.tile([128, 512], f32, tag="qkv",
                                         name=f"ap{j}_{hf}")
                        o = hf * 512
                        nc.tensor.matmul(ap[:, :], was(j),
                                         xtj[:, o:o + 512])
                        if hf % 2 == 0:
                            nc.scalar.copy(As[j][:, o:o + 512], ap[:, :])
                        else:
                            nc.vector.tensor_copy(As[j][:, o:o + 512],
                                                  ap[:, :])
                    Bs[j] = ab_pool.tile([128, S], bf16, tag="ab",
                                         name=f"B{j}")
                    nc.sync.dma_start(out=Bs[j][0:64, :],
                                      in_=As[j][64:128, :])
                    nc.sync.dma_start(out=Bs[j][64:128, :],
                                      in_=As[j][0:64, :])

                def emit_v(j):
                    xtj = xts[j]
                    # V psums: 7 tiles of 65 cols per 512-col bank so no
                    # matmul output crosses a PSUM bank boundary.
                    Vs[j] = v_pool.tile([128, NT * (D + 1)], bf16,
                                        tag="v", name=f"V{j}")
                    vps = [qkv_ps.tile([128, 512], f32, tag="qkv",
                                       name=f"vp{j}_{b}") for b in range(3)]
                    for tt in range(NT):
                        vp = vps[tt // 7]
                        dst = vp[:, (tt % 7) * 65:(tt % 7) * 65 + 65]
                        nc.tensor.matmul(
                            dst, xtj[:, tt * 128:(tt + 1) * 128], wvs(j))
                    for bk in range(3):
                        nb = min(7, NT - bk * 7)
                        dst = Vs[j][:, bk * 7 * (D + 1):
                                    (bk * 7 + nb) * (D + 1)]
                        src = vps[bk][:, 0:nb * 65]
                        # alternate engines per block so each head's
                        # three V-copies split across ScalarE and DVE
                        if (bk + j) % 2 == 0:
                            nc.vector.tensor_copy(dst, src)
                        else:
                            nc.scalar.copy(dst, src)

                # All A matmuls first: dense N=512 streams un-throttle
                # the PE HAM clock gate early; the V block follows.
                for j in range(HL):
                    emit_a(j)
                for j in range(HL):
                    emit_v(j)

            # ---- phase 2: attention, global 3-ahead pipeline ----
            with (
                tc.tile_pool(name="psR", bufs=3, space="PSUM") as psR,
                tc.tile_pool(name="psav", bufs=2, space="PSUM") as psav,
                tc.tile_pool(name="eg", bufs=8) as eg_pool,
                tc.tile_pool(name="egh", bufs=10) as egh_pool,
                tc.tile_pool(name="eh", bufs=3) as eh_pool,
                tc.tile_pool(name="ot", bufs=2) as ot_pool,
                tc.tile_pool(name="norm", bufs=3) as norm_pool,
            ):
                NCH = HL * NC_CHUNK          # 32 global chunks
                K = NCH * NU                 # 256 global units

                oTs = {}                     # head -> oT tile
                avs = {}                     # chunk -> av tile
                egs = {}                     # unit -> (eg_lo_ap, eg_hi_ap)
                avn = {}                     # chunk -> accumulated count
                csd = {}
                bcd = {}
                sched = {}                   # global unit idx -> [closures]

                def at(k, fn):
                    sched.setdefault(k, []).append(fn)

                def fill_unit(k):
                    j, r = divmod(k, NC_CHUNK * NU)
                    c, u = divmod(r, NU)
                    sl = slice(c * 512, (c + 1) * 512)
                    gt = psR.tile([128, 1024], f32, tag="psR",
                                  name=f"g{k}")
                    ta, tb = 2 * u, 2 * u + 1
                    nc.tensor.matmul(
                        gt[:, 0:512],
                        Bs[j][0:64, ta * 128:(ta + 1) * 128],
                        As[j][0:64, sl], tile_position=(0, 0))
                    nc.tensor.matmul(
                        gt[:, 512:1024],
                        As[j][64:128, tb * 128:(tb + 1) * 128],
                        Bs[j][64:128, sl], tile_position=(64, 0))
                    return gt

                def emit_exp(k, gt):
                    j, r = divmod(k, NC_CHUNK * NU)
                    c, u = divmod(r, NU)
                    if EU[u] == "act":
                        eg = eg_pool.tile([128, 1024], bf16, tag="eg",
                                          name=f"eg{k}")
                        nc.scalar.activation(
                            eg[:, :], gt[:, :],
                            mybir.ActivationFunctionType.Exp,
                            scale=ACT_SCALE)
                        egs[k] = (eg[:, 0:512], eg[:, 512:1024])
                    else:
                        eh = eh_pool.tile([128, 1024], bf16, tag="eh",
                                          name=f"eh{k}")
                        nc.vector._custom_dve(
                            exp4, out=eh[:, :], in0=gt[:, :],
                            s0=EXP4_C3, s1=EXP4_C2, imm2=EXP4_C1)
                        elo = egh_pool.tile([128, 512], bf16,
                                            tag="egh", name=f"egl{k}")
                        ehi = egh_pool.tile([128, 512], bf16,
                                            tag="egh", name=f"egh{k}")
                        nc.vector.tensor_tensor(
                            elo[:, :], eh[:, 0:512], eh[:, 0:512],
                            op=mybir.AluOpType.mult)
                        nc.gpsimd.tensor_tensor(
                            ehi[:, :], eh[:, 512:1024], eh[:, 512:1024],
                            op=mybir.AluOpType.mult)
                        egs[k] = (elo[:, :], ehi[:, :])

                def emit_av(k):
                    j, r = divmod(k, NC_CHUNK * NU)
                    c, u = divmod(r, NU)
                    m = k // NU              # global chunk
                    if m not in avs:
                        avs[m] = psav.tile([D + 1, 512], f32, tag="psav",
                                           name=f"av{m}")
                        avn[m] = 0
                    av = avs[m]
                    eg_lo, eg_hi = egs.pop(k)
                    for half, egp in ((0, eg_lo), (1, eg_hi)):
                        tt = 2 * u + half
                        nc.tensor.matmul(
                            av[:, :],
                            Vs[j][:, tt * (D + 1):(tt + 1) * (D + 1)],
                            egp,
                            start=(avn[m] == 0),
                            stop=(avn[m] == NT - 1))
                        avn[m] += 1

                def queue_chunk_end(m):
                    """Called right after emit_av completes chunk m, at
                    global unit k0 = (m+1)*NU + LAG - 1."""
                    j, c = divmod(m, NC_CHUNK)
                    k0 = (m + 1) * NU + LAG - 1
                    if j == HL - 1:
                        # last head: fast per-chunk norm path (chunk-sized
                        # denominator round trips so the tail is short)
                        emit_evict(m)
                        emit_cs(j, c * 512, 512)
                        if c > 0:
                            s0 = (c - 1) * 512
                            at(k0 + 1,
                               lambda: emit_recip_bcast(j, s0, 512))
                            at(k0 + 3, lambda: emit_mult(j, s0, s0, 512))
                        return
                    at(k0 + 2, lambda: emit_evict(m))
                    if c % 2 == 1:
                        h0 = (c - 1) * 512
                        at(k0 + 3, lambda: emit_cs(j, h0, 1024))
                        at(k0 + 6, lambda: emit_recip_bcast(j, h0, 1024))
                        at(k0 + 14, lambda: emit_mult(j, h0, h0, 512))
                        at(k0 + 16,
                           lambda: emit_mult(j, h0, h0 + 512, 512))

                def emit_evict(m):
                    j, c = divmod(m, NC_CHUNK)
                    if j not in oTs:
                        oTs[j] = ot_pool.tile([D + 1, S], f32, tag="ot",
                                              name=f"oT{j}")
                    av = avs.pop(m)
                    del avn[m]
                    if m % 2 == 0:
                        nc.vector.tensor_copy(
                            oTs[j][:, c * 512:(c + 1) * 512], av[:, :])
                    else:
                        nc.scalar.copy(
                            oTs[j][:, c * 512:(c + 1) * 512], av[:, :])

                def emit_cs(j, s0, n):
                    cs = norm_pool.tile([128, 16], f32, tag="cs",
                                        name=f"cs{j}_{s0}")
                    nc.sync.dma_start(out=cs[:, 0:n // 128],
                                      in_=oTs[j][D:D + 1, s0:s0 + n])
                    csd[(j, s0)] = cs

                def emit_recip_bcast(j, s0, n):
                    cs = csd.pop((j, s0))
                    rc = norm_pool.tile([128, 16], f32, tag="rc",
                                        name=f"rc{j}_{s0}")
                    nc.vector.reciprocal(rc[:, 0:n // 128], cs[:, 0:n // 128])
                    nc.sync.dma_start(out=recip_d[j, s0:s0 + n],
                                      in_=rc[:, 0:n // 128])
                    bc = norm_pool.tile([D, 1024], f32, tag="bc",
                                        name=f"bc{j}_{s0}")
                    nc.sync.dma_start(
                        out=bc[:, 0:n],
                        in_=recip_d[j, s0:s0 + n].unsqueeze(0)
                        .broadcast_to((D, n)))
                    bcd[(j, s0)] = bc

                def emit_mult(j, h0, s0, n):
                    bc = bcd[(j, h0)]
                    boff = s0 - h0
                    ct = cts[j // 2]
                    if j % 2 == 1:
                        # odd head: ct rows 0:64, direct GpSimd write
                        nc.gpsimd.tensor_tensor(
                            ct[0:D, s0:s0 + n],
                            oTs[j][0:D, s0:s0 + n],
                            bc[:, boff:boff + n],
                            op=mybir.AluOpType.mult)
                    else:
                        # even head: bounce via [64,*] tile + SBUF DMA
                        # into ct rows 64:128 (partition shift)
                        dst = norm_pool.tile([D, 1024], bf16, tag="ctmp",
                                             name=f"ctmp{j}_{s0}")
                        nc.gpsimd.tensor_tensor(
                            dst[:, 0:n],
                            oTs[j][0:D, s0:s0 + n],
                            bc[:, boff:boff + n],
                            op=mybir.AluOpType.mult)
                        nc.sync.dma_start(
                            out=ct[D:2 * D, s0:s0 + n], in_=dst[:, 0:n])

                for k in range(K + LAG):
                    if k < K:
                        gt = fill_unit(k)
                        emit_exp(k, gt)
                    ka = k - LAG
                    if ka >= 0:
                        emit_av(ka)
                        if ka % NU == NU - 1:
                            queue_chunk_end(ka // NU)
                    for fn in sched.pop(k, []):
                        fn()

                # drain remaining scheduled work, then the final chunk's
                # normalization for the last head.
                for k in sorted(sched):
                    for fn in sched[k]:
                        fn()
                sched.clear()
                j = HL - 1
                emit_recip_bcast(j, 3 * 512, 512)
                emit_mult(j, 3 * 512, 3 * 512, 512)

            # ---- phase 3: output projection ----
            with (
                tc.tile_pool(name="pj_ps", bufs=4, space="PSUM") as pj_ps,
                tc.tile_pool(name="po", bufs=4) as po_pool,
            ):
                NS = S // 128
                psos = {}

                def pj_front(sc):
                    pso = pj_ps.tile([128, E], f32, tag="pj",
                                     name=f"pso{sc}")
                    psos[sc] = pso
                    for p in range(3):
                        for half in range(2):
                            hsl = slice(half * 512, (half + 1) * 512)
                            nc.tensor.matmul(
                                pso[:, hsl],
                                cts[p][:, sc * 128:(sc + 1) * 128],
                                wot_t[:, p * E + half * 512:
                                      p * E + (half + 1) * 512],
                                start=(p == 0), stop=False)

                def pj_back(sc):
                    pso = psos.pop(sc)
                    for half in range(2):
                        hsl = slice(half * 512, (half + 1) * 512)
                        nc.tensor.matmul(
                            pso[:, hsl],
                            cts[3][:, sc * 128:(sc + 1) * 128],
                            wot_t[:, 3 * E + half * 512:
                                  3 * E + (half + 1) * 512],
                            start=False, stop=True)
                    # bf16 partial output: halves the out-DMA traffic.
                    # Separate even/odd tile tags keep the vector and
                    # scalar copy chains independent.
                    if sc % 2 == 0:
                        osb = po_pool.tile([128, E], bf16, tag="pov")
                        nc.vector.tensor_copy(osb[:, :], pso[:, :])
                    else:
                        osb = po_pool.tile([128, E], bf16, tag="pos")
                        nc.scalar.copy(osb[:, :], pso[:, :])
                    nc.sync.dma_start(out=out_d[sc * 128:(sc + 1) * 128, :],
                                      in_=osb[:, :])

                for sc in range(NS + 4):
                    if sc < NS:
                        pj_front(sc)
                    if sc >= 4:
                        pj_back(sc - 4)

    nc.compile()
    return nc


def prep_inputs(token_encodings, Wq, Wk, Wv, bq, bk, bv, Wo, bo):
    """Build per-core input maps. Core c = b*2+g."""
    x = np.asarray(token_encodings, dtype=np.float32)
    wq = np.asarray(Wq, np.float32)
    wk = np.asarray(Wk, np.float32)
    wv = np.asarray(Wv, np.float32)
    bq_ = np.asarray(bq, np.float32)
    bk_ = np.asarray(bk, np.float32)
    bv_ = np.asarray(bv, np.float32)
    wo = np.asarray(Wo, np.float32)
    maps = []
    for c in range(N_CORES):
        b, g = divmod(c, 2)
        xt_full = np.ascontiguousarray(x[b].T)  # (E, S)
        xt = np.zeros((HL, 128, S), dtype=BF16)
        WSTR = 256
        wqkv = np.zeros((128, HL * WSTR), dtype=BF16)
        for j in range(HL):
            h = g * HL + j
            xt[j, :D] = xt_full[h * D:(h + 1) * D].astype(BF16)
            xt[j, D] = np.float32(1.0)
            o = j * WSTR
            # A-stationary: cols 0:64 -> Q' (scaled), cols 64:128 -> K
            wqkv[:D, o:o + D] = (wq[h] * SIG).astype(BF16)
            wqkv[D, o:o + D] = (bq_[h] * SIG).astype(BF16)
            wqkv[:D, o + D:o + 2 * D] = wk[h].astype(BF16)
            wqkv[D, o + D:o + 2 * D] = bk_[h].astype(BF16)
            # V-stationary: cols 128:192 -> V, col 192 -> denominator
            wqkv[:D, o + 128:o + 128 + D] = wv[h].astype(BF16)
            wqkv[D, o + 128:o + 128 + D] = bv_[h].astype(BF16)
            wqkv[D, o + 128 + D] = np.float32(1.0)
        # wot rows permuted: ct[p] rows 0:64 = head 2p+1, 64:128 = head 2p
        order = []
        for p in range(4):
            for r in range(128):
                j_loc = 2 * p + 1 if r < 64 else 2 * p
                order.append(g * 512 + j_loc * D + (r % D))
        wot = np.ascontiguousarray(wo[:, order].T).astype(BF16)
        maps.append({"xt": xt, "wqkv": wqkv, "wot": wot})
    return maps


def kernel(**inputs):
    from concourse.bass_utils import run_bass_kernel_spmd

    if "nc" not in _CACHE:
        _CACHE["nc"] = build_nc()
    nc = _CACHE["nc"]
    in_maps = prep_inputs(**inputs)
    res = run_bass_kernel_spmd(nc, in_maps, list(range(N_CORES)))
    bo_f = np.asarray(inputs["bo"], np.float32)
    out = np.empty((B, S, E), dtype=np.float32)
    for b in range(B):
        out[b] = (res.results[2 * b]["out"].astype(np.float32)
                  + res.results[2 * b + 1]["out"].astype(np.float32)
                  + bo_f)
    return out
